# revision 41
# baseline (speedup 1.0000x reference)
"""Trainium2 Bass kernel for the isotropic-gaussian differentiable renderer.

Math: for pixel p=(x,y) and gaussian g:
    w[g,p] = op_g * exp(-0.5*((x-ax_g)^2+(y-ay_g)^2)/var_g)
    img[p,c] = (sum_g w[g,p]*col_gc) / (sum_g w[g,p] + n_chunks*EPS)

The isotropic RBF is separable: w = op * exp(sx) * exp(sy) with
sx = s*(x-ax)^2, sy = s*(y-ay)^2 + ln(op), s = -0.5/var.  That turns the
268M-element exp into 2*N*128 exps plus matmuls:

  per 128-gaussian chunk:
    PE (f32r): arg[g, 0:128]=sx(g,x), arg[g,128:256]=sy(g,y) via a K=12
               matmul against fixed rows [u^2,u,1|v^2,v,1] duplicated for a
               hi/lo coefficient split (centered coords; the split keeps the
               catastrophically-cancelling quadratic exact in f32r)
    ACT      : exp(arg) -> fp16 written into fused per-chunk blocks
               [expx(128) | B(128) | colors(384)]; the y half lands as the
               den block B = op*expy directly (ln(op) is in the argument)
    DVE      : 3 tensor_scalar ops fill the color blocks col_c*B from the
               SAME rounded B, so fp16 weight rounding cancels in num/den
    PE (fp16): acc[x, (den|c)*128+y] += block[0:128]^T @ block[128:640]
               (fp32 PSUM accumulate; PE pre-warmed off memset tiles so
               f32r arg matmuls stream at 1 cyc/row)

Sharding: gaussians split 2048/core across 8 cores; every core accumulates
the full 128x128 image; host sums the 8 partials, divides num/den and
reshapes to the reference's [4,3,64,64] tile layout.
"""
import numpy as np

import concourse.bacc as bacc
import concourse.tile as tile
from concourse import mybir
from concourse.bass_utils import run_bass_kernel_spmd

# Problem constants (hardcoded per harness contract)
N_GAUSS = 16384
H = 128
W = 128
FX = 128.0
FY = 128.0
CX = 64.0
CY = 64.0
EPS = 1e-8
N_CORES = 8
G_PER_CORE = N_GAUSS // N_CORES      # 2048
CHUNK = 128                          # gaussians per matmul chunk
N_CHUNKS = G_PER_CORE // CHUNK       # 16
ARG_W = 256                          # per-chunk arg width: 128 x | 128 y
GROUP = 4                            # chunks per exp batch
N_GROUPS = N_CHUNKS // GROUP         # 4
OUT_W = 512                          # (c,y) free width of the accumulator

F32 = mybir.dt.float32
MM_DT = mybir.dt.float8e4            # main-accumulation matmul dtype.
# fp8e4 is safe for the same reason fp16 was: expx rounding cancels
# exactly between num and den (same lhsT), B rounding mostly cancels
# (colB multiplies the rounded B), and the residual per-gaussian color
# rounding (~6%) averages out over the ~10^3 gaussians per pixel.
# Measured scale-relative error ~7e-3 vs the 2e-2 gate.
F32R = mybir.dt.float32r
KARG = 14                            # arg-matmul contraction: 7 bf16 coef rows/axis
PACK = 4                             # arg matmuls packed per PE pass (row groups)
USE_PACK = False                     # tile_position matmuls crash TRN2 here; keep off


def build_program():
    """One SPMD Bass program; every core runs it on its gaussian slice."""
    nc = bacc.Bacc("TRN2", target_bir_lowering=False, debug=False,
                   num_devices=N_CORES)
    # packed: [128, 4*128]: coefpack[32k+r, grp*128+j] = coef row r of chunk
    # (grp*PACK+k), gaussian j — four chunks stacked at partition 0/32/64/96
    # so four K=6 arg matmuls run concurrently in separate PE row groups.
    # unpacked: [6, 2048] flat, one chunk per 128 columns.
    # coef rows for all chunks plus the fixed moving rows, packed into ONE
    # dram tensor: every engine-issued dma_start costs ~0.6-1.1us of serial
    # descriptor generation on its sequencer, so the latency-critical arg
    # inputs must be a single transfer.
    cpack_shape = [KARG, G_PER_CORE + ARG_W]
    cpack = nc.dram_tensor("cpack", cpack_shape, mybir.dt.bfloat16,
                           kind="ExternalInput")
    # [128, 16, 3]: colnarrow[p, chunk, c] = col_c[chunk*128+p] (bf16) —
    # broadcast on-chip along y by the DVE color multiply, so the whole
    # color input is one tiny DMA instead of a 1.5MB replicated tensor.
    colnarrow = nc.dram_tensor("colnarrow", [128, N_CHUNKS, 3],
                               mybir.dt.bfloat16, kind="ExternalInput")
    # partial accumulator: [x, c*128+y], fp16 (halves the output DMA; the
    # host sums the 8 per-core partials in float64 anyway)
    out = nc.dram_tensor("out", [128, OUT_W], mybir.dt.float16,
                         kind="ExternalOutput")

    with tile.TileContext(nc) as tc:
        with tc.tile_pool(name="ins", bufs=1) as ins_pool, \
             tc.tile_pool(name="expp", bufs=1) as exp_pool, \
             tc.tile_pool(name="args", bufs=3, space="PSUM") as arg_pool, \
             tc.tile_pool(name="acc", bufs=1, space="PSUM") as acc_pool, \
             tc.tile_pool(name="warmp", bufs=1, space="PSUM") as warm_pool, \
             tc.tile_pool(name="outp", bufs=1) as out_pool:

            cpack_t = ins_pool.tile(cpack_shape, mybir.dt.bfloat16)
            coln_t = ins_pool.tile([128, N_CHUNKS, 3], mybir.dt.bfloat16)
            # The hardware DMA rings don't move data until ~8.4us into the
            # kernel no matter when the doorbells ring (runtime init floor),
            # and GpSimd's DIRECT2D path is slower still — so everything
            # rides the sync ring, latency-critical coef first, then the
            # big colrep in 2-chunk slices so chunk c's colors land just
            # ahead of its DVE multiply.
            wsrc = ins_pool.tile([128, ARG_W], mybir.dt.bfloat16)
            nc.gpsimd.memset(wsrc, 0.0)
            # sync ring: just two small input transfers (~76KB total), so
            # every consumer's DMA wait clears by ~9us.
            nc.sync.dma_start(out=cpack_t, in_=cpack[:, :])
            nc.sync.dma_start(out=coln_t, in_=colnarrow[:, :])
            # ACT's scale operand must be f32: one cheap DVE copy converts
            # the 6 entries its chunks need (first in the DVE stream; its
            # only dependency is the tiny colnarrow DMA).
            opcf = ins_pool.tile([128, 2, 3, 1], F32)
            nc.vector.tensor_copy(opcf, coln_t[:, 14:16, :].unsqueeze(3))


            # plane-major storage [plane, chunk, y]: planes are
            # [expx | B | r | g | b].  The DoubleRow lhsT (expx plane)
            # stays chunk-packed, and ONE activation per group writes the
            # x and B planes together through a transposed AP.
            exall = exp_pool.tile([128, 5, N_CHUNKS, 128], MM_DT)
            acc = acc_pool.tile([128, OUT_W], F32)

            # PE warmup off the memset tile (ready ~6.5us, while inputs
            # stream in): dummy matmuls flip the HAM clock gate toward 8/8
            # so the real arg matmuls stream at 1 cyc/row, in dead time.
            wdst = warm_pool.tile([128, ARG_W], F32)
            for _ in range(8):
                nc.tensor.matmul(wdst[:, :], wsrc[:, :CHUNK], wsrc[:, :],
                                 start=True, stop=True)

            # narrow leading groups tighten the pipeline front: chunk 0's
            # A-build waits on a 1-chunk exp instead of a 4-chunk batch
            group_plan = [(0, 2), (2, 2), (4, 4), (8, 4), (12, 4)]
            for g0c, width in group_plan:
                args = arg_pool.tile([128, width, ARG_W], F32, tag="args")
                for k in range(width):
                    chunk = g0c + k
                    nc.tensor.matmul(
                        args[:, k, :],
                        cpack_t[:, chunk * CHUNK:(chunk + 1) * CHUNK],
                        cpack_t[:, G_PER_CORE:G_PER_CORE + ARG_W],
                        start=True, stop=True,
                    )
                # one call writes [x|B] for the whole group: out iterates
                # (chunk, plane, y) to match the psum args layout
                nc.scalar.activation(
                    out=exall[:, 0:2, g0c:g0c + width, :].transpose(
                        [0, 2, 1, 3]),
                    in_=args[:, :, :],
                    func=mybir.ActivationFunctionType.Exp,
                )

            # color blocks multiply the SAME rounded B so num/den rounding
            # cancels.  rhs column order: [den(B)|r|g|b].  DVE: ONE fused
            # tensor_mul per chunk PAIR (B broadcast across the 3 channels,
            # colnarrow broadcast along y).  The last pair runs on ACT
            # (Copy with per-partition scale), which ends its exp stream
            # about when DVE finishes pair 6 — the engines drain the color
            # work in parallel.
            for p in range(7):
                nc.vector.tensor_mul(
                    exall[:, 2:5, 2 * p:2 * p + 2, :],
                    exall[:, 1:2, 2 * p:2 * p + 2, :].broadcast_to(
                        [128, 3, 2, 128]),
                    coln_t[:, 2 * p:2 * p + 2, :].transpose(
                        [0, 2, 1]).unsqueeze(3).broadcast_to(
                        [128, 3, 2, 128]),
                )
            for chunk in (14, 15):
                for c in range(3):
                    nc.scalar.mul(
                        exall[:, 2 + c, chunk, :],
                        exall[:, 1, chunk, :],
                        opcf[:, chunk - 14, c],
                    )
            # fp8 DoubleRow: one matmul contracts a PAIR of 128-gaussian
            # chunks (2 k-subtiles, packed strides).  Column-split into
            # [den|r] and [g|b] halves so the first half's PSUM drain +
            # output DMA overlaps the second half's matmuls.
            # full-width DoubleRow mains (interleaving two accumulation
            # groups into one PSUM tile corrupts the result — keep one
            # group).  PSUM drains as two fp16 copies back-to-back on ACT;
            # both output doorbells ride sync so the copies aren't split
            # by the slow on-engine descriptor generation.
            N_PAIRS = N_CHUNKS // 2
            out_t = out_pool.tile([128, OUT_W], mybir.dt.float16)
            pair_order = [0, 1, 2, 3, 4, 5, 7, 6]  # 6=(12,13) DVE-last
            for i, p in enumerate(pair_order):
                nc.tensor.matmul(
                    acc[:, :],
                    exall[:, 0, 2 * p:2 * p + 2, :],
                    exall[:, 1:5, 2 * p:2 * p + 2, :].transpose(
                        [0, 2, 1, 3]),
                    start=(i == 0), stop=(i == N_PAIRS - 1),
                    perf_mode=mybir.MatmulPerfMode.DoubleRow,
                )
            nc.vector.tensor_copy(out_t[:, :256], acc[:, :256])
            nc.sync.dma_start(out=out[:, :256], in_=out_t[:, :256])
            nc.scalar.copy(out=out_t[:, 256:], in_=acc[:, 256:])
            nc.scalar.dma_start(out=out[:, 256:], in_=out_t[:, 256:])

    nc.compile()
    return nc


_PROGRAM = None


def _get_program():
    global _PROGRAM
    if _PROGRAM is None:
        _PROGRAM = build_program()
    return _PROGRAM


def _quat2mat(q):
    q = q / np.linalg.norm(q)
    w, x, y, z = q
    return np.array([
        [1 - 2 * (y * y + z * z), 2 * (x * y - z * w), 2 * (x * z + y * w)],
        [2 * (x * y + z * w), 1 - 2 * (x * x + z * z), 2 * (y * z - x * w)],
        [2 * (x * z - y * w), 2 * (y * z + x * w), 1 - 2 * (x * x + y * y)],
    ])


def kernel(positions, colors, opacities, scales, qvec, tvec, tile_hw,
           chunk_gauss, _trace=False):
    positions = np.asarray(positions, dtype=np.float32)
    colors = np.asarray(colors, dtype=np.float32)
    opacities = np.asarray(opacities, dtype=np.float32)
    scales = np.asarray(scales, dtype=np.float32)
    qvec = np.asarray(qvec, dtype=np.float32)
    tvec = np.asarray(tvec, dtype=np.float32)
    tile_hw = int(tile_hw)
    chunk_gauss = int(chunk_gauss)
    n = positions.shape[0]
    assert n == N_GAUSS, f"expected {N_GAUSS} gaussians, got {n}"

    # ---- O(N) per-gaussian prep in float64 (rounds to the same f32 values
    # the reference computes, to well within the exp's own error budget) ----
    R = _quat2mat(qvec.astype(np.float64))
    cam = positions.astype(np.float64) @ R.T + tvec.astype(np.float64)
    ax = cam[:, 0] / cam[:, 2] * FX + CX          # [N] screen x center
    ay = cam[:, 1] / cam[:, 2] * FY + CY          # [N] screen y center
    var = scales[:, 0].astype(np.float64) ** 2
    s = -0.5 / var                                # [N] negative inv 2*var

    # centered coords keep the quadratic-expansion terms small (|u|<=64)
    dx = ax - CX
    dy = ay - CY

    bf16 = mybir.dt.np(mybir.dt.bfloat16)

    def hilo(x):
        """Split x into bf16 hi+lo with hi+lo ~= x to ~2^-17 relative."""
        hi = np.asarray(x, np.float32).astype(bf16)
        lo = (np.asarray(x, np.float64) - hi.astype(np.float64)
              ).astype(np.float32).astype(bf16)
        return hi, lo

    # K=7 bf16 stationary rows per axis per gaussian, for
    #   arg_x = s*u^2 + (-2 s dx)*u + s*dx^2     (u = x - 64)
    # expanded so every PE product is (near-)exact despite the bf16
    # 8-bit significand: s*u^2 = shi*u2hi + shi*u2lo + slo*u2hi (+O(2^-16)),
    # where u2hi+u2lo is an EXACT bf16 split of u^2 (u2lo is a small int).
    # The linear/const terms use plain hi/lo pairs against exact rows.
    # +ln(op) on the y-constant row makes exp(arg_y) = op*exp_y directly.
    # Max |arg error| ~4e-3 -> ~0.4% worst-case weight error, averages out.
    op64 = opacities[:, 0].astype(np.float64)
    axes = [(s, -2.0 * s * dx, s * dx * dx),
            (s, -2.0 * s * dy, s * dy * dy + np.log(op64))]
    coef_rows = []
    for (a, b, c) in axes:
        ahi, alo = hilo(a)
        bhi, blo = hilo(b)
        chi, clo = hilo(c)
        coef_rows.extend([ahi, ahi, alo, bhi, blo, chi, clo])
    coef_full = np.stack(coef_rows)                       # [14, N] bf16

    u = np.arange(W, dtype=np.float64) - CX
    u2 = u * u
    u2hi = u2.astype(np.float32).astype(bf16)
    u2lo = (u2 - u2hi.astype(np.float64)).astype(np.float32).astype(bf16)
    assert np.all(u2hi.astype(np.float64) + u2lo.astype(np.float64) == u2)
    zeros = np.zeros(128, dtype=bf16)
    ones = np.ones(128, dtype=bf16)
    ub = u.astype(np.float32).astype(bf16)
    axis_rows = [u2hi, u2lo, u2hi, ub, ub, ones, ones]
    rhs_rows = []
    for r in axis_rows:
        rhs_rows.append(np.concatenate([r, zeros]))
    for r in axis_rows:
        rhs_rows.append(np.concatenate([zeros, r]))
    rhsxy = np.stack(rhs_rows)                            # [14, 256] bf16

    # colnarrow[p, chunk, c] = col_c[chunk*128+p] (bf16): the DVE color
    # multiply broadcasts it along y on-chip.
    col16 = colors.astype(np.float32).astype(bf16)        # [N, 3]

    # ---- shard gaussians across the 8 cores ----
    in_maps = []
    for core in range(N_CORES):
        g0 = core * G_PER_CORE
        g1 = g0 + G_PER_CORE
        coln_c = np.ascontiguousarray(
            col16[g0:g1].reshape(N_CHUNKS, CHUNK, 3).transpose(1, 0, 2))
        cpack_c = np.ascontiguousarray(
            np.concatenate([coef_full[:, g0:g1], rhsxy], axis=1))
        in_maps.append({
            "cpack": cpack_c,
            "colnarrow": coln_c,
        })

    nc = _get_program()
    res = run_bass_kernel_spmd(nc, in_maps, list(range(N_CORES)),
                               trace=_trace)

    # ---- host reduction: sum per-core partials, divide, reshape ----
    acc = np.zeros((128, 4, 128), dtype=np.float64)   # [x, (den|r|g|b), y]
    for core in range(N_CORES):
        acc += res.results[core]["out"].astype(np.float64).reshape(128, 4, 128)

    num = acc[:, 1:4, :]                          # [x, c, y]
    n_chunks_ref = n // chunk_gauss
    den = acc[:, 0, :] + n_chunks_ref * EPS       # [x, y]
    img = num / den[:, None, :]                   # [x, c, y]
    img = img.transpose(2, 0, 1).reshape(H * W, 3)  # [p=(y,x), c]

    step = tile_hw * tile_hw
    t = (H * W) // step
    out = img.reshape(t, step, 3).transpose(0, 2, 1).reshape(
        t, 3, tile_hw, tile_hw)
    result = out.astype(np.float32)
    if _trace:
        return result, res
    return result



# revision 42
# speedup vs baseline: 1.0008x; 1.0008x over previous
"""Trainium2 Bass kernel for the isotropic-gaussian differentiable renderer.

Math: for pixel p=(x,y) and gaussian g:
    w[g,p] = op_g * exp(-0.5*((x-ax_g)^2+(y-ay_g)^2)/var_g)
    img[p,c] = (sum_g w[g,p]*col_gc) / (sum_g w[g,p] + n_chunks*EPS)

The isotropic RBF is separable: w = op * exp(sx) * exp(sy) with
sx = s*(x-ax)^2, sy = s*(y-ay)^2 + ln(op), s = -0.5/var.  That turns the
268M-element exp into 2*N*128 exps plus matmuls:

  per 128-gaussian chunk:
    PE (bf16): arg[g, 0:128]=sx(g,x), arg[g,128:256]=sy(g,y) via a K=14
               matmul against fixed rows [u2hi,u2lo,u2hi,u,u,1,1|...] with
               bf16 hi/lo coefficient splits (centered coords keep every
               product near-exact despite the cancelling quadratic;
               max |arg err| ~4e-3).  bf16 needs NO on-chip cast, so the
               args start the moment the single coef DMA lands.
    ACT      : ONE exp per group writes the x|B planes of the plane-major
               fp8 tile exall[128, 5, chunk, y] (planes expx|B|r|g|b);
               B = op*expy directly (ln(op) is in the argument)
    DVE      : one fused tensor_mul per chunk PAIR fills the r|g|b planes
               col_c*B (colnarrow broadcast along y on-chip); the LAST
               pair runs on ACT via Copy-with-scale so both engines drain
               the color work in parallel.  All consume the SAME rounded
               fp8 B, so weight rounding cancels in num/den.
    PE (fp8) : 8 DoubleRow matmuls, each contracting a chunk PAIR
               (2 k-subtiles, 2 rows/cycle):
               acc[x, (den|c)*128+y] += expx_pair^T @ [B|r|g|b]_pair
               (fp32 PSUM accumulate; PE pre-warmed off memset tiles).

Scheduling notes (hard-won):
  - engine-issued dma_start costs 0.6-1.1us of SERIAL descriptor
    generation on its sequencer, and rings deliver no data before ~8.4us:
    all inputs are packed into 2 small transfers (cpack 64KB + colnarrow
    12KB) on the sync ring.
  - interleaving two PSUM accumulation groups in one tile corrupts
    results; the mains form a single full-width group.
  - GpSimd fp8 ops and gpsimd tensor_copy crash or run ~10x slow; only
    memset rides there.

Sharding: gaussians split 2048/core across 8 cores; every core accumulates
the full 128x128 image; host sums the 8 fp16 partials in float64, divides
num/den and reshapes to the reference's [4,3,64,64] tile layout.
Measured: ~23.5us HW exec, scale-rel err ~6.8e-3 (gate 2e-2).
"""
import numpy as np

import concourse.bacc as bacc
import concourse.tile as tile
from concourse import mybir
from concourse.bass_utils import run_bass_kernel_spmd

# Problem constants (hardcoded per harness contract)
N_GAUSS = 16384
H = 128
W = 128
FX = 128.0
FY = 128.0
CX = 64.0
CY = 64.0
EPS = 1e-8
N_CORES = 8
G_PER_CORE = N_GAUSS // N_CORES      # 2048
CHUNK = 128                          # gaussians per matmul chunk
N_CHUNKS = G_PER_CORE // CHUNK       # 16
ARG_W = 256                          # per-chunk arg width: 128 x | 128 y
GROUP = 4                            # chunks per exp batch
N_GROUPS = N_CHUNKS // GROUP         # 4
OUT_W = 512                          # (c,y) free width of the accumulator

F32 = mybir.dt.float32
MM_DT = mybir.dt.float8e4            # main-accumulation matmul dtype.
# fp8e4 is safe for the same reason fp16 was: expx rounding cancels
# exactly between num and den (same lhsT), B rounding mostly cancels
# (colB multiplies the rounded B), and the residual per-gaussian color
# rounding (~6%) averages out over the ~10^3 gaussians per pixel.
# Measured scale-relative error ~7e-3 vs the 2e-2 gate.
KARG = 14                            # arg-matmul contraction: 7 bf16 coef rows/axis


def build_program():
    """One SPMD Bass program; every core runs it on its gaussian slice."""
    nc = bacc.Bacc("TRN2", target_bir_lowering=False, debug=False,
                   num_devices=N_CORES)
    # coef rows for all chunks plus the fixed moving rows, packed into ONE
    # dram tensor: every engine-issued dma_start costs ~0.6-1.1us of serial
    # descriptor generation on its sequencer, so the latency-critical arg
    # inputs must be a single transfer.
    cpack_shape = [KARG, G_PER_CORE + ARG_W]
    cpack = nc.dram_tensor("cpack", cpack_shape, mybir.dt.bfloat16,
                           kind="ExternalInput")
    # [128, 16, 3]: colnarrow[p, chunk, c] = col_c[chunk*128+p] (bf16) —
    # broadcast on-chip along y by the DVE color multiply, so the whole
    # color input is one tiny DMA instead of a 1.5MB replicated tensor.
    colnarrow = nc.dram_tensor("colnarrow", [128, N_CHUNKS, 3],
                               mybir.dt.bfloat16, kind="ExternalInput")
    # partial accumulator: [x, c*128+y], fp16 (halves the output DMA; the
    # host sums the 8 per-core partials in float64 anyway)
    out = nc.dram_tensor("out", [128, OUT_W], mybir.dt.float16,
                         kind="ExternalOutput")

    with tile.TileContext(nc) as tc:
        with tc.tile_pool(name="ins", bufs=1) as ins_pool, \
             tc.tile_pool(name="expp", bufs=1) as exp_pool, \
             tc.tile_pool(name="args", bufs=3, space="PSUM") as arg_pool, \
             tc.tile_pool(name="acc", bufs=1, space="PSUM") as acc_pool, \
             tc.tile_pool(name="warmp", bufs=1, space="PSUM") as warm_pool, \
             tc.tile_pool(name="outp", bufs=1) as out_pool:

            cpack_t = ins_pool.tile(cpack_shape, mybir.dt.bfloat16)
            coln_t = ins_pool.tile([128, N_CHUNKS, 3], mybir.dt.bfloat16)
            # The hardware DMA rings don't move data until ~8.4us into the
            # kernel no matter when the doorbells ring (runtime init floor),
            # and GpSimd's DIRECT2D path is slower still — so everything
            # rides the sync ring, latency-critical coef first, then the
            # big colrep in 2-chunk slices so chunk c's colors land just
            # ahead of its DVE multiply.
            wsrc = ins_pool.tile([128, ARG_W], mybir.dt.bfloat16)
            nc.gpsimd.memset(wsrc, 0.0)
            # sync ring: just two small input transfers (~76KB total), so
            # every consumer's DMA wait clears by ~9us.
            nc.sync.dma_start(out=cpack_t, in_=cpack[:, :])
            nc.sync.dma_start(out=coln_t, in_=colnarrow[:, :])
            # ACT's scale operand must be f32: one cheap DVE copy converts
            # the 6 entries its chunks need (first in the DVE stream; its
            # only dependency is the tiny colnarrow DMA).
            opcf = ins_pool.tile([128, 2, 3, 1], F32)
            nc.vector.tensor_copy(opcf, coln_t[:, 14:16, :].unsqueeze(3))


            # plane-major storage [plane, chunk, y]: planes are
            # [expx | B | r | g | b].  The DoubleRow lhsT (expx plane)
            # stays chunk-packed, and ONE activation per group writes the
            # x and B planes together through a transposed AP.
            exall = exp_pool.tile([128, 5, N_CHUNKS, 128], MM_DT)
            acc = acc_pool.tile([128, OUT_W], F32)

            # PE warmup off the memset tile (ready ~6.5us, while inputs
            # stream in): dummy matmuls flip the HAM clock gate toward 8/8
            # so the real arg matmuls stream at 1 cyc/row, in dead time.
            wdst = warm_pool.tile([128, ARG_W], F32)
            for _ in range(8):
                nc.tensor.matmul(wdst[:, :], wsrc[:, :CHUNK], wsrc[:, :],
                                 start=True, stop=True)

            # narrow leading groups tighten the pipeline front: chunk 0's
            # A-build waits on a 1-chunk exp instead of a 4-chunk batch
            group_plan = [(0, 2), (2, 2), (4, 4), (8, 4), (12, 4)]
            for g0c, width in group_plan:
                args = arg_pool.tile([128, width, ARG_W], F32, tag="args")
                for k in range(width):
                    chunk = g0c + k
                    nc.tensor.matmul(
                        args[:, k, :],
                        cpack_t[:, chunk * CHUNK:(chunk + 1) * CHUNK],
                        cpack_t[:, G_PER_CORE:G_PER_CORE + ARG_W],
                        start=True, stop=True,
                    )
                # one call writes [x|B] for the whole group: out iterates
                # (chunk, plane, y) to match the psum args layout
                nc.scalar.activation(
                    out=exall[:, 0:2, g0c:g0c + width, :].transpose(
                        [0, 2, 1, 3]),
                    in_=args[:, :, :],
                    func=mybir.ActivationFunctionType.Exp,
                )

            # color blocks multiply the SAME rounded B so num/den rounding
            # cancels.  rhs column order: [den(B)|r|g|b].  DVE: ONE fused
            # tensor_mul per chunk PAIR (B broadcast across the 3 channels,
            # colnarrow broadcast along y).  The last pair runs on ACT
            # (Copy with per-partition scale), which ends its exp stream
            # about when DVE finishes pair 6 — the engines drain the color
            # work in parallel.
            for p in range(7):
                nc.vector.tensor_mul(
                    exall[:, 2:5, 2 * p:2 * p + 2, :],
                    exall[:, 1:2, 2 * p:2 * p + 2, :].broadcast_to(
                        [128, 3, 2, 128]),
                    coln_t[:, 2 * p:2 * p + 2, :].transpose(
                        [0, 2, 1]).unsqueeze(3).broadcast_to(
                        [128, 3, 2, 128]),
                )
            for chunk in (14, 15):
                for c in range(3):
                    nc.scalar.mul(
                        exall[:, 2 + c, chunk, :],
                        exall[:, 1, chunk, :],
                        opcf[:, chunk - 14, c],
                    )
            # fp8 DoubleRow: one matmul contracts a PAIR of 128-gaussian
            # chunks (2 k-subtiles, packed strides).  Column-split into
            # [den|r] and [g|b] halves so the first half's PSUM drain +
            # output DMA overlaps the second half's matmuls.
            # full-width DoubleRow mains (interleaving two accumulation
            # groups into one PSUM tile corrupts the result — keep one
            # group).  PSUM drains as two fp16 copies back-to-back on ACT;
            # both output doorbells ride sync so the copies aren't split
            # by the slow on-engine descriptor generation.
            N_PAIRS = N_CHUNKS // 2
            out_t = out_pool.tile([128, OUT_W], mybir.dt.float16)
            pair_order = [0, 1, 2, 3, 4, 5, 7, 6]  # 6=(12,13) DVE-last
            for i, p in enumerate(pair_order):
                nc.tensor.matmul(
                    acc[:, :],
                    exall[:, 0, 2 * p:2 * p + 2, :],
                    exall[:, 1:5, 2 * p:2 * p + 2, :].transpose(
                        [0, 2, 1, 3]),
                    start=(i == 0), stop=(i == N_PAIRS - 1),
                    perf_mode=mybir.MatmulPerfMode.DoubleRow,
                )
            nc.vector.tensor_copy(out_t[:, :256], acc[:, :256])
            nc.sync.dma_start(out=out[:, :256], in_=out_t[:, :256])
            nc.scalar.copy(out=out_t[:, 256:], in_=acc[:, 256:])
            nc.scalar.dma_start(out=out[:, 256:], in_=out_t[:, 256:])

    nc.compile()
    return nc


_PROGRAM = None


def _get_program():
    global _PROGRAM
    if _PROGRAM is None:
        _PROGRAM = build_program()
    return _PROGRAM


def _quat2mat(q):
    q = q / np.linalg.norm(q)
    w, x, y, z = q
    return np.array([
        [1 - 2 * (y * y + z * z), 2 * (x * y - z * w), 2 * (x * z + y * w)],
        [2 * (x * y + z * w), 1 - 2 * (x * x + z * z), 2 * (y * z - x * w)],
        [2 * (x * z - y * w), 2 * (y * z + x * w), 1 - 2 * (x * x + y * y)],
    ])


def kernel(positions, colors, opacities, scales, qvec, tvec, tile_hw,
           chunk_gauss, _trace=False):
    positions = np.asarray(positions, dtype=np.float32)
    colors = np.asarray(colors, dtype=np.float32)
    opacities = np.asarray(opacities, dtype=np.float32)
    scales = np.asarray(scales, dtype=np.float32)
    qvec = np.asarray(qvec, dtype=np.float32)
    tvec = np.asarray(tvec, dtype=np.float32)
    tile_hw = int(tile_hw)
    chunk_gauss = int(chunk_gauss)
    n = positions.shape[0]
    assert n == N_GAUSS, f"expected {N_GAUSS} gaussians, got {n}"

    # ---- O(N) per-gaussian prep in float64 (rounds to the same f32 values
    # the reference computes, to well within the exp's own error budget) ----
    R = _quat2mat(qvec.astype(np.float64))
    cam = positions.astype(np.float64) @ R.T + tvec.astype(np.float64)
    ax = cam[:, 0] / cam[:, 2] * FX + CX          # [N] screen x center
    ay = cam[:, 1] / cam[:, 2] * FY + CY          # [N] screen y center
    var = scales[:, 0].astype(np.float64) ** 2
    s = -0.5 / var                                # [N] negative inv 2*var

    # centered coords keep the quadratic-expansion terms small (|u|<=64)
    dx = ax - CX
    dy = ay - CY

    bf16 = mybir.dt.np(mybir.dt.bfloat16)

    def hilo(x):
        """Split x into bf16 hi+lo with hi+lo ~= x to ~2^-17 relative."""
        hi = np.asarray(x, np.float32).astype(bf16)
        lo = (np.asarray(x, np.float64) - hi.astype(np.float64)
              ).astype(np.float32).astype(bf16)
        return hi, lo

    # K=7 bf16 stationary rows per axis per gaussian, for
    #   arg_x = s*u^2 + (-2 s dx)*u + s*dx^2     (u = x - 64)
    # expanded so every PE product is (near-)exact despite the bf16
    # 8-bit significand: s*u^2 = shi*u2hi + shi*u2lo + slo*u2hi (+O(2^-16)),
    # where u2hi+u2lo is an EXACT bf16 split of u^2 (u2lo is a small int).
    # The linear/const terms use plain hi/lo pairs against exact rows.
    # +ln(op) on the y-constant row makes exp(arg_y) = op*exp_y directly.
    # Max |arg error| ~4e-3 -> ~0.4% worst-case weight error, averages out.
    op64 = opacities[:, 0].astype(np.float64)
    axes = [(s, -2.0 * s * dx, s * dx * dx),
            (s, -2.0 * s * dy, s * dy * dy + np.log(op64))]
    coef_rows = []
    for (a, b, c) in axes:
        ahi, alo = hilo(a)
        bhi, blo = hilo(b)
        chi, clo = hilo(c)
        coef_rows.extend([ahi, ahi, alo, bhi, blo, chi, clo])
    coef_full = np.stack(coef_rows)                       # [14, N] bf16

    u = np.arange(W, dtype=np.float64) - CX
    u2 = u * u
    u2hi = u2.astype(np.float32).astype(bf16)
    u2lo = (u2 - u2hi.astype(np.float64)).astype(np.float32).astype(bf16)
    assert np.all(u2hi.astype(np.float64) + u2lo.astype(np.float64) == u2)
    zeros = np.zeros(128, dtype=bf16)
    ones = np.ones(128, dtype=bf16)
    ub = u.astype(np.float32).astype(bf16)
    axis_rows = [u2hi, u2lo, u2hi, ub, ub, ones, ones]
    rhs_rows = []
    for r in axis_rows:
        rhs_rows.append(np.concatenate([r, zeros]))
    for r in axis_rows:
        rhs_rows.append(np.concatenate([zeros, r]))
    rhsxy = np.stack(rhs_rows)                            # [14, 256] bf16

    # colnarrow[p, chunk, c] = col_c[chunk*128+p] (bf16): the DVE color
    # multiply broadcasts it along y on-chip.
    col16 = colors.astype(np.float32).astype(bf16)        # [N, 3]

    # ---- shard gaussians across the 8 cores ----
    in_maps = []
    for core in range(N_CORES):
        g0 = core * G_PER_CORE
        g1 = g0 + G_PER_CORE
        coln_c = np.ascontiguousarray(
            col16[g0:g1].reshape(N_CHUNKS, CHUNK, 3).transpose(1, 0, 2))
        cpack_c = np.ascontiguousarray(
            np.concatenate([coef_full[:, g0:g1], rhsxy], axis=1))
        in_maps.append({
            "cpack": cpack_c,
            "colnarrow": coln_c,
        })

    nc = _get_program()
    res = run_bass_kernel_spmd(nc, in_maps, list(range(N_CORES)),
                               trace=_trace)

    # ---- host reduction: sum per-core partials, divide, reshape ----
    acc = np.zeros((128, 4, 128), dtype=np.float64)   # [x, (den|r|g|b), y]
    for core in range(N_CORES):
        acc += res.results[core]["out"].astype(np.float64).reshape(128, 4, 128)

    num = acc[:, 1:4, :]                          # [x, c, y]
    n_chunks_ref = n // chunk_gauss
    den = acc[:, 0, :] + n_chunks_ref * EPS       # [x, y]
    img = num / den[:, None, :]                   # [x, c, y]
    img = img.transpose(2, 0, 1).reshape(H * W, 3)  # [p=(y,x), c]

    step = tile_hw * tile_hw
    t = (H * W) // step
    out = img.reshape(t, step, 3).transpose(0, 2, 1).reshape(
        t, 3, tile_hw, tile_hw)
    result = out.astype(np.float32)
    if _trace:
        return result, res
    return result



# revision 43
# speedup vs baseline: 1.0038x; 1.0030x over previous
"""Trainium2 Bass kernel for the isotropic-gaussian differentiable renderer.

Math: for pixel p=(x,y) and gaussian g:
    w[g,p] = op_g * exp(-0.5*((x-ax_g)^2+(y-ay_g)^2)/var_g)
    img[p,c] = (sum_g w[g,p]*col_gc) / (sum_g w[g,p] + n_chunks*EPS)

The isotropic RBF is separable: w = op * exp(sx) * exp(sy) with
sx = s*(x-ax)^2, sy = s*(y-ay)^2 + ln(op), s = -0.5/var.  That turns the
268M-element exp into 2*N*128 exps plus matmuls:

  per 128-gaussian chunk:
    PE (bf16): arg[g, 0:128]=sx(g,x), arg[g,128:256]=sy(g,y) via a K=14
               matmul against fixed rows [u2hi,u2lo,u2hi,u,u,1,1|...] with
               bf16 hi/lo coefficient splits (centered coords keep every
               product near-exact despite the cancelling quadratic;
               max |arg err| ~4e-3).  bf16 needs NO on-chip cast, so the
               args start the moment the single coef DMA lands.
    ACT      : ONE exp per group writes the x|B planes of the plane-major
               fp8 tile exall[128, 5, chunk, y] (planes expx|B|r|g|b);
               B = op*expy directly (ln(op) is in the argument)
    DVE      : one fused tensor_mul per chunk PAIR fills the r|g|b planes
               col_c*B (colnarrow broadcast along y on-chip); the LAST
               pair runs on ACT via Copy-with-scale so both engines drain
               the color work in parallel.  All consume the SAME rounded
               fp8 B, so weight rounding cancels in num/den.
    PE (fp8) : 8 DoubleRow matmuls, each contracting a chunk PAIR
               (2 k-subtiles, 2 rows/cycle):
               acc[x, (den|c)*128+y] += expx_pair^T @ [B|r|g|b]_pair
               (fp32 PSUM accumulate; PE pre-warmed off memset tiles).

Scheduling notes (hard-won):
  - engine-issued dma_start costs 0.6-1.1us of SERIAL descriptor
    generation on its sequencer, and rings deliver no data before ~8.4us:
    all inputs are packed into 2 small transfers (cpack 64KB + colnarrow
    12KB) on the sync ring.
  - interleaving two PSUM accumulation groups in one tile corrupts
    results; the mains form a single full-width group.
  - GpSimd fp8 ops and gpsimd tensor_copy crash or run ~10x slow; only
    memset rides there.

Sharding: gaussians split 2048/core across 8 cores; every core accumulates
the full 128x128 image; host sums the 8 fp16 partials in float64, divides
num/den and reshapes to the reference's [4,3,64,64] tile layout.
Measured: ~23.5us HW exec, scale-rel err ~6.8e-3 (gate 2e-2).
"""
import numpy as np

import concourse.bacc as bacc
import concourse.tile as tile
from concourse import mybir
from concourse.bass_utils import run_bass_kernel_spmd

# Problem constants (hardcoded per harness contract)
N_GAUSS = 16384
H = 128
W = 128
FX = 128.0
FY = 128.0
CX = 64.0
CY = 64.0
EPS = 1e-8
N_CORES = 8
G_PER_CORE = N_GAUSS // N_CORES      # 2048
CHUNK = 128                          # gaussians per matmul chunk
N_CHUNKS = G_PER_CORE // CHUNK       # 16
ARG_W = 256                          # per-chunk arg width: 128 x | 128 y
GROUP = 4                            # chunks per exp batch
N_GROUPS = N_CHUNKS // GROUP         # 4
OUT_W = 512                          # (c,y) free width of the accumulator

F32 = mybir.dt.float32
MM_DT = mybir.dt.float8e4            # main-accumulation matmul dtype.
# fp8e4 is safe for the same reason fp16 was: expx rounding cancels
# exactly between num and den (same lhsT), B rounding mostly cancels
# (colB multiplies the rounded B), and the residual per-gaussian color
# rounding (~6%) averages out over the ~10^3 gaussians per pixel.
# Measured scale-relative error ~7e-3 vs the 2e-2 gate.
KARG = 14                            # arg-matmul contraction: 7 bf16 coef rows/axis


def build_program():
    """One SPMD Bass program; every core runs it on its gaussian slice."""
    nc = bacc.Bacc("TRN2", target_bir_lowering=False, debug=False,
                   num_devices=N_CORES)
    # coef rows for all chunks plus the fixed moving rows, packed into ONE
    # dram tensor: every engine-issued dma_start costs ~0.6-1.1us of serial
    # descriptor generation on its sequencer, so the latency-critical arg
    # inputs must be a single transfer.
    cpack_shape = [KARG, G_PER_CORE + ARG_W]
    cpack = nc.dram_tensor("cpack", cpack_shape, mybir.dt.bfloat16,
                           kind="ExternalInput")
    # [128, 16, 3]: colnarrow[p, chunk, c] = col_c[chunk*128+p] (bf16) —
    # broadcast on-chip along y by the DVE color multiply, so the whole
    # color input is one tiny DMA instead of a 1.5MB replicated tensor.
    colnarrow = nc.dram_tensor("colnarrow", [128, N_CHUNKS, 3],
                               mybir.dt.bfloat16, kind="ExternalInput")
    # partial accumulator: [x, c*128+y], fp16 (halves the output DMA; the
    # host sums the 8 per-core partials in float64 anyway)
    out = nc.dram_tensor("out", [128, OUT_W], mybir.dt.float16,
                         kind="ExternalOutput")

    with tile.TileContext(nc) as tc:
        with tc.tile_pool(name="ins", bufs=1) as ins_pool, \
             tc.tile_pool(name="expp", bufs=1) as exp_pool, \
             tc.tile_pool(name="args", bufs=3, space="PSUM") as arg_pool, \
             tc.tile_pool(name="acc", bufs=1, space="PSUM") as acc_pool, \
             tc.tile_pool(name="warmp", bufs=1, space="PSUM") as warm_pool, \
             tc.tile_pool(name="outp", bufs=1) as out_pool:

            cpack_t = ins_pool.tile(cpack_shape, mybir.dt.bfloat16)
            coln_t = ins_pool.tile([128, N_CHUNKS, 3], mybir.dt.bfloat16)
            # warmup operand, memset on the early-starting GpSimd
            wsrc = ins_pool.tile([128, ARG_W], mybir.dt.bfloat16)
            nc.gpsimd.memset(wsrc, 0.0)
            # sync ring: just two small input transfers (~76KB total), so
            # every consumer's DMA wait clears by ~9us.
            nc.sync.dma_start(out=cpack_t, in_=cpack[:, :])
            nc.sync.dma_start(out=coln_t, in_=colnarrow[:, :])
            # ACT's scale operand must be f32: one cheap DVE copy converts
            # the 6 entries its chunks need (first in the DVE stream; its
            # only dependency is the tiny colnarrow DMA).
            opcf = ins_pool.tile([128, 2, 3, 1], F32)
            nc.vector.tensor_copy(opcf, coln_t[:, 14:16, :].unsqueeze(3))


            # plane-major storage [plane, chunk, y]: planes are
            # [expx | B | r | g | b].  The DoubleRow lhsT (expx plane)
            # stays chunk-packed, and ONE activation per group writes the
            # x and B planes together through a transposed AP.
            exall = exp_pool.tile([128, 5, N_CHUNKS, 128], MM_DT)
            acc = acc_pool.tile([128, OUT_W], F32)

            # PE warmup off the memset tile (ready ~6.5us, while inputs
            # stream in): dummy matmuls flip the HAM clock gate toward 8/8
            # so the real arg matmuls stream at 1 cyc/row, in dead time.
            wdst = warm_pool.tile([128, ARG_W], F32)
            for _ in range(8):
                nc.tensor.matmul(wdst[:, :], wsrc[:, :CHUNK], wsrc[:, :],
                                 start=True, stop=True)

            # narrow leading groups tighten the pipeline front: chunk 0's
            # A-build waits on a 1-chunk exp instead of a 4-chunk batch
            group_plan = [(0, 2), (2, 2), (4, 4), (8, 4), (12, 4)]
            for g0c, width in group_plan:
                args = arg_pool.tile([128, width, ARG_W], F32, tag="args")
                for k in range(width):
                    chunk = g0c + k
                    nc.tensor.matmul(
                        args[:, k, :],
                        cpack_t[:, chunk * CHUNK:(chunk + 1) * CHUNK],
                        cpack_t[:, G_PER_CORE:G_PER_CORE + ARG_W],
                        start=True, stop=True,
                    )
                # one call writes [x|B] for the whole group: out iterates
                # (chunk, plane, y) to match the psum args layout
                nc.scalar.activation(
                    out=exall[:, 0:2, g0c:g0c + width, :].transpose(
                        [0, 2, 1, 3]),
                    in_=args[:, :, :],
                    func=mybir.ActivationFunctionType.Exp,
                )

            # color blocks multiply the SAME rounded B so num/den rounding
            # cancels.  rhs column order: [den(B)|r|g|b].  DVE: ONE fused
            # tensor_mul per chunk PAIR (B broadcast across the 3 channels,
            # colnarrow broadcast along y).  The last pair runs on ACT
            # (Copy with per-partition scale), which ends its exp stream
            # about when DVE finishes pair 6 — the engines drain the color
            # work in parallel.
            for p in range(7):
                nc.vector.tensor_mul(
                    exall[:, 2:5, 2 * p:2 * p + 2, :],
                    exall[:, 1:2, 2 * p:2 * p + 2, :].broadcast_to(
                        [128, 3, 2, 128]),
                    coln_t[:, 2 * p:2 * p + 2, :].transpose(
                        [0, 2, 1]).unsqueeze(3).broadcast_to(
                        [128, 3, 2, 128]),
                )
            for chunk in (14, 15):
                for c in range(3):
                    nc.scalar.mul(
                        exall[:, 2 + c, chunk, :],
                        exall[:, 1, chunk, :],
                        opcf[:, chunk - 14, c],
                    )
            # fp8 DoubleRow mains: one matmul contracts a PAIR of
            # 128-gaussian chunks (2 k-subtiles at 2 rows/cycle).  Full
            # width, single accumulation group (interleaving two groups in
            # one PSUM tile corrupts the result).  The PSUM then drains as
            # two fp16 copies on DVE+ACT in parallel, with the two output
            # doorbells on separate rings (sync/scalar) so their ~0.7us
            # descriptor generations also overlap.
            N_PAIRS = N_CHUNKS // 2
            out_t = out_pool.tile([128, OUT_W], mybir.dt.float16)
            pair_order = [0, 1, 2, 3, 4, 5, 7, 6]  # 6=(12,13) DVE-last
            for i, p in enumerate(pair_order):
                nc.tensor.matmul(
                    acc[:, :],
                    exall[:, 0, 2 * p:2 * p + 2, :],
                    exall[:, 1:5, 2 * p:2 * p + 2, :].transpose(
                        [0, 2, 1, 3]),
                    start=(i == 0), stop=(i == N_PAIRS - 1),
                    perf_mode=mybir.MatmulPerfMode.DoubleRow,
                )
            nc.vector.tensor_copy(out_t[:, :256], acc[:, :256])
            nc.sync.dma_start(out=out[:, :256], in_=out_t[:, :256])
            nc.scalar.copy(out=out_t[:, 256:], in_=acc[:, 256:])
            nc.scalar.dma_start(out=out[:, 256:], in_=out_t[:, 256:])

    nc.compile()
    return nc


_PROGRAM = None


def _get_program():
    global _PROGRAM
    if _PROGRAM is None:
        _PROGRAM = build_program()
    return _PROGRAM


def _quat2mat(q):
    q = q / np.linalg.norm(q)
    w, x, y, z = q
    return np.array([
        [1 - 2 * (y * y + z * z), 2 * (x * y - z * w), 2 * (x * z + y * w)],
        [2 * (x * y + z * w), 1 - 2 * (x * x + z * z), 2 * (y * z - x * w)],
        [2 * (x * z - y * w), 2 * (y * z + x * w), 1 - 2 * (x * x + y * y)],
    ])


def kernel(positions, colors, opacities, scales, qvec, tvec, tile_hw,
           chunk_gauss, _trace=False):
    positions = np.asarray(positions, dtype=np.float32)
    colors = np.asarray(colors, dtype=np.float32)
    opacities = np.asarray(opacities, dtype=np.float32)
    scales = np.asarray(scales, dtype=np.float32)
    qvec = np.asarray(qvec, dtype=np.float32)
    tvec = np.asarray(tvec, dtype=np.float32)
    tile_hw = int(tile_hw)
    chunk_gauss = int(chunk_gauss)
    n = positions.shape[0]
    assert n == N_GAUSS, f"expected {N_GAUSS} gaussians, got {n}"

    # ---- O(N) per-gaussian prep in float64 (rounds to the same f32 values
    # the reference computes, to well within the exp's own error budget) ----
    R = _quat2mat(qvec.astype(np.float64))
    cam = positions.astype(np.float64) @ R.T + tvec.astype(np.float64)
    ax = cam[:, 0] / cam[:, 2] * FX + CX          # [N] screen x center
    ay = cam[:, 1] / cam[:, 2] * FY + CY          # [N] screen y center
    var = scales[:, 0].astype(np.float64) ** 2
    s = -0.5 / var                                # [N] negative inv 2*var

    # centered coords keep the quadratic-expansion terms small (|u|<=64)
    dx = ax - CX
    dy = ay - CY

    bf16 = mybir.dt.np(mybir.dt.bfloat16)

    def hilo(x):
        """Split x into bf16 hi+lo with hi+lo ~= x to ~2^-17 relative."""
        hi = np.asarray(x, np.float32).astype(bf16)
        lo = (np.asarray(x, np.float64) - hi.astype(np.float64)
              ).astype(np.float32).astype(bf16)
        return hi, lo

    # K=7 bf16 stationary rows per axis per gaussian, for
    #   arg_x = s*u^2 + (-2 s dx)*u + s*dx^2     (u = x - 64)
    # expanded so every PE product is (near-)exact despite the bf16
    # 8-bit significand: s*u^2 = shi*u2hi + shi*u2lo + slo*u2hi (+O(2^-16)),
    # where u2hi+u2lo is an EXACT bf16 split of u^2 (u2lo is a small int).
    # The linear/const terms use plain hi/lo pairs against exact rows.
    # +ln(op) on the y-constant row makes exp(arg_y) = op*exp_y directly.
    # Max |arg error| ~4e-3 -> ~0.4% worst-case weight error, averages out.
    op64 = opacities[:, 0].astype(np.float64)
    axes = [(s, -2.0 * s * dx, s * dx * dx),
            (s, -2.0 * s * dy, s * dy * dy + np.log(op64))]
    coef_rows = []
    for (a, b, c) in axes:
        ahi, alo = hilo(a)
        bhi, blo = hilo(b)
        chi, clo = hilo(c)
        coef_rows.extend([ahi, ahi, alo, bhi, blo, chi, clo])
    coef_full = np.stack(coef_rows)                       # [14, N] bf16

    u = np.arange(W, dtype=np.float64) - CX
    u2 = u * u
    u2hi = u2.astype(np.float32).astype(bf16)
    u2lo = (u2 - u2hi.astype(np.float64)).astype(np.float32).astype(bf16)
    assert np.all(u2hi.astype(np.float64) + u2lo.astype(np.float64) == u2)
    zeros = np.zeros(128, dtype=bf16)
    ones = np.ones(128, dtype=bf16)
    ub = u.astype(np.float32).astype(bf16)
    axis_rows = [u2hi, u2lo, u2hi, ub, ub, ones, ones]
    rhs_rows = []
    for r in axis_rows:
        rhs_rows.append(np.concatenate([r, zeros]))
    for r in axis_rows:
        rhs_rows.append(np.concatenate([zeros, r]))
    rhsxy = np.stack(rhs_rows)                            # [14, 256] bf16

    # colnarrow[p, chunk, c] = col_c[chunk*128+p] (bf16): the DVE color
    # multiply broadcasts it along y on-chip.
    col16 = colors.astype(np.float32).astype(bf16)        # [N, 3]

    # ---- shard gaussians across the 8 cores ----
    in_maps = []
    for core in range(N_CORES):
        g0 = core * G_PER_CORE
        g1 = g0 + G_PER_CORE
        coln_c = np.ascontiguousarray(
            col16[g0:g1].reshape(N_CHUNKS, CHUNK, 3).transpose(1, 0, 2))
        cpack_c = np.ascontiguousarray(
            np.concatenate([coef_full[:, g0:g1], rhsxy], axis=1))
        in_maps.append({
            "cpack": cpack_c,
            "colnarrow": coln_c,
        })

    nc = _get_program()
    res = run_bass_kernel_spmd(nc, in_maps, list(range(N_CORES)),
                               trace=_trace)

    # ---- host reduction: sum per-core partials, divide, reshape ----
    acc = np.zeros((128, 4, 128), dtype=np.float64)   # [x, (den|r|g|b), y]
    for core in range(N_CORES):
        acc += res.results[core]["out"].astype(np.float64).reshape(128, 4, 128)

    num = acc[:, 1:4, :]                          # [x, c, y]
    n_chunks_ref = n // chunk_gauss
    den = acc[:, 0, :] + n_chunks_ref * EPS       # [x, y]
    img = num / den[:, None, :]                   # [x, c, y]
    img = img.transpose(2, 0, 1).reshape(H * W, 3)  # [p=(y,x), c]

    step = tile_hw * tile_hw
    t = (H * W) // step
    out = img.reshape(t, step, 3).transpose(0, 2, 1).reshape(
        t, 3, tile_hw, tile_hw)
    result = out.astype(np.float32)
    if _trace:
        return result, res
    return result



# revision 44
# speedup vs baseline: 1.0163x; 1.0125x over previous
"""Trainium2 Bass kernel for the isotropic-gaussian differentiable renderer.

Math: for pixel p=(x,y) and gaussian g:
    w[g,p] = op_g * exp(-0.5*((x-ax_g)^2+(y-ay_g)^2)/var_g)
    img[p,c] = (sum_g w[g,p]*col_gc) / (sum_g w[g,p] + n_chunks*EPS)

The isotropic RBF is separable: w = op * exp(sx) * exp(sy) with
sx = s*(x-ax)^2, sy = s*(y-ay)^2 + ln(op), s = -0.5/var.  That turns the
268M-element exp into 2*N*128 exps plus matmuls:

  per 128-gaussian chunk:
    PE (bf16): arg[g, 0:128]=sx(g,x), arg[g,128:256]=sy(g,y) via a K=14
               matmul against fixed rows [u2hi,u2lo,u2hi,u,u,1,1|...] with
               bf16 hi/lo coefficient splits (centered coords keep every
               product near-exact despite the cancelling quadratic;
               max |arg err| ~4e-3).  bf16 needs NO on-chip cast, so the
               args start the moment the single coef DMA lands.
    ACT      : ONE exp per group writes the x|B planes of the plane-major
               fp8 tile exall[128, 5, chunk, y] (planes expx|B|r|g|b);
               B = op*expy directly (ln(op) is in the argument)
    DVE      : one fused tensor_mul per chunk PAIR fills the r|g|b planes
               col_c*B (colnarrow broadcast along y on-chip); the LAST
               pair runs on ACT via Copy-with-scale so both engines drain
               the color work in parallel.  All consume the SAME rounded
               fp8 B, so weight rounding cancels in num/den.
    PE (fp8) : 8 DoubleRow matmuls, each contracting a chunk PAIR
               (2 k-subtiles, 2 rows/cycle):
               acc[x, (den|c)*128+y] += expx_pair^T @ [B|r|g|b]_pair
               (fp32 PSUM accumulate; PE pre-warmed off memset tiles).

Scheduling notes (hard-won):
  - engine-issued dma_start costs 0.6-1.1us of SERIAL descriptor
    generation on its sequencer, and rings deliver no data before ~8.4us:
    all inputs are packed into 2 small transfers (cpack 64KB + colnarrow
    12KB) on the sync ring.
  - interleaving two PSUM accumulation groups in one tile corrupts
    results; the mains form a single full-width group.
  - GpSimd fp8 ops and gpsimd tensor_copy crash or run ~10x slow; only
    memset rides there.

Sharding: gaussians split 2048/core across 8 cores; every core accumulates
the full 128x128 image; host sums the 8 fp16 partials in float64, divides
num/den and reshapes to the reference's [4,3,64,64] tile layout.
Measured: ~23.5us HW exec, scale-rel err ~6.8e-3 (gate 2e-2).
"""
import numpy as np

import concourse.bacc as bacc
import concourse.tile as tile
from concourse import mybir
from concourse.bass_utils import run_bass_kernel_spmd

# Problem constants (hardcoded per harness contract)
N_GAUSS = 16384
H = 128
W = 128
FX = 128.0
FY = 128.0
CX = 64.0
CY = 64.0
EPS = 1e-8
N_CORES = 8
G_PER_CORE = N_GAUSS // N_CORES      # 2048
CHUNK = 128                          # gaussians per matmul chunk
N_CHUNKS = G_PER_CORE // CHUNK       # 16
ARG_W = 256                          # per-chunk arg width: 128 x | 128 y
GROUP = 4                            # chunks per exp batch
N_GROUPS = N_CHUNKS // GROUP         # 4
OUT_W = 512                          # (c,y) free width of the accumulator

F32 = mybir.dt.float32
MM_DT = mybir.dt.float8e4            # main-accumulation matmul dtype.
# fp8e4 is safe for the same reason fp16 was: expx rounding cancels
# exactly between num and den (same lhsT), B rounding mostly cancels
# (colB multiplies the rounded B), and the residual per-gaussian color
# rounding (~6%) averages out over the ~10^3 gaussians per pixel.
# Measured scale-relative error ~7e-3 vs the 2e-2 gate.
KARG = 14                            # arg-matmul contraction: 7 bf16 coef rows/axis


def build_program():
    """One SPMD Bass program; every core runs it on its gaussian slice."""
    nc = bacc.Bacc("TRN2", target_bir_lowering=False, debug=False,
                   num_devices=N_CORES)
    # coef rows for all chunks plus the fixed moving rows, packed into ONE
    # dram tensor: every engine-issued dma_start costs ~0.6-1.1us of serial
    # descriptor generation on its sequencer, so the latency-critical arg
    # inputs must be a single transfer.
    cpack_shape = [KARG, G_PER_CORE + ARG_W]
    cpack = nc.dram_tensor("cpack", cpack_shape, mybir.dt.bfloat16,
                           kind="ExternalInput")
    # [128, 16, 3]: colnarrow[p, chunk, c] = col_c[chunk*128+p] (bf16) —
    # broadcast on-chip along y by the DVE color multiply, so the whole
    # color input is one tiny DMA instead of a 1.5MB replicated tensor.
    colnarrow = nc.dram_tensor("colnarrow", [128, N_CHUNKS, 3],
                               mybir.dt.bfloat16, kind="ExternalInput")
    # partial accumulator: [x, c*128+y], fp16 (halves the output DMA; the
    # host sums the 8 per-core partials in float64 anyway)
    out = nc.dram_tensor("out", [128, OUT_W], mybir.dt.float16,
                         kind="ExternalOutput")

    with tile.TileContext(nc) as tc:
        with tc.tile_pool(name="ins", bufs=1) as ins_pool, \
             tc.tile_pool(name="expp", bufs=1) as exp_pool, \
             tc.tile_pool(name="args", bufs=3, space="PSUM") as arg_pool, \
             tc.tile_pool(name="acc", bufs=1, space="PSUM") as acc_pool, \
             tc.tile_pool(name="warmp", bufs=1, space="PSUM") as warm_pool, \
             tc.tile_pool(name="outp", bufs=1) as out_pool:

            cpack_t = ins_pool.tile(cpack_shape, mybir.dt.bfloat16)
            coln_t = ins_pool.tile([128, N_CHUNKS, 3], mybir.dt.bfloat16)
            # warmup operand, memset on the early-starting GpSimd
            wsrc = ins_pool.tile([128, ARG_W], mybir.dt.bfloat16)
            nc.gpsimd.memset(wsrc, 0.0)
            # sync ring: just two small input transfers (~76KB total), so
            # every consumer's DMA wait clears by ~9us.
            nc.sync.dma_start(out=cpack_t, in_=cpack[:, :])
            nc.sync.dma_start(out=coln_t, in_=colnarrow[:, :])
            # ACT's scale operand must be f32: one cheap DVE copy converts
            # the 6 entries its chunks need (first in the DVE stream; its
            # only dependency is the tiny colnarrow DMA).
            opcf = ins_pool.tile([128, 2, 3, 1], F32)
            nc.vector.tensor_copy(opcf, coln_t[:, 14:16, :].unsqueeze(3))


            # plane-major storage [plane, chunk, y]: planes are
            # [expx | B | r | g | b].  The DoubleRow lhsT (expx plane)
            # stays chunk-packed, and ONE activation per group writes the
            # x and B planes together through a transposed AP.
            exall = exp_pool.tile([128, 5, N_CHUNKS, 128], MM_DT)
            acc = acc_pool.tile([128, OUT_W], F32)

            # PE warmup off the memset tile (ready ~6.5us, while inputs
            # stream in): dummy matmuls flip the HAM clock gate toward 8/8
            # so the real arg matmuls stream at 1 cyc/row, in dead time.
            wdst = warm_pool.tile([128, ARG_W], F32)
            for _ in range(9):
                nc.tensor.matmul(wdst[:, :], wsrc[:, :CHUNK], wsrc[:, :],
                                 start=True, stop=True)

            # narrow leading groups tighten the pipeline front: chunk 0's
            # A-build waits on a 1-chunk exp instead of a 4-chunk batch
            group_plan = [(0, 1), (1, 1), (2, 2), (4, 4), (8, 4), (12, 4)]
            for g0c, width in group_plan:
                args = arg_pool.tile([128, width, ARG_W], F32, tag="args")
                for k in range(width):
                    chunk = g0c + k
                    nc.tensor.matmul(
                        args[:, k, :],
                        cpack_t[:, chunk * CHUNK:(chunk + 1) * CHUNK],
                        cpack_t[:, G_PER_CORE:G_PER_CORE + ARG_W],
                        start=True, stop=True,
                    )
                # one call writes [x|B] for the whole group: out iterates
                # (chunk, plane, y) to match the psum args layout
                nc.scalar.activation(
                    out=exall[:, 0:2, g0c:g0c + width, :].transpose(
                        [0, 2, 1, 3]),
                    in_=args[:, :, :],
                    func=mybir.ActivationFunctionType.Exp,
                )

            # color blocks multiply the SAME rounded B so num/den rounding
            # cancels.  rhs column order: [den(B)|r|g|b].  DVE: ONE fused
            # tensor_mul per chunk PAIR (B broadcast across the 3 channels,
            # colnarrow broadcast along y).  The last pair runs on ACT
            # (Copy with per-partition scale), which ends its exp stream
            # about when DVE finishes pair 6 — the engines drain the color
            # work in parallel.
            # pair 0 as two per-chunk ops so the DVE stream starts the
            # moment chunk 0's exp lands (the pair op would wait chunk 1)
            for c0, cw in [(0, 1), (1, 1), (2, 2), (4, 2), (6, 2),
                           (8, 2), (10, 2), (12, 2)]:
                nc.vector.tensor_mul(
                    exall[:, 2:5, c0:c0 + cw, :],
                    exall[:, 1:2, c0:c0 + cw, :].broadcast_to(
                        [128, 3, cw, 128]),
                    coln_t[:, c0:c0 + cw, :].transpose(
                        [0, 2, 1]).unsqueeze(3).broadcast_to(
                        [128, 3, cw, 128]),
                )
            for chunk in (14, 15):
                for c in range(3):
                    nc.scalar.mul(
                        exall[:, 2 + c, chunk, :],
                        exall[:, 1, chunk, :],
                        opcf[:, chunk - 14, c],
                    )
            # fp8 DoubleRow mains: one matmul contracts a PAIR of
            # 128-gaussian chunks (2 k-subtiles at 2 rows/cycle).  Full
            # width, single accumulation group (interleaving two groups in
            # one PSUM tile corrupts the result).  The PSUM then drains as
            # two fp16 copies on DVE+ACT in parallel, with the two output
            # doorbells on separate rings (sync/scalar) so their ~0.7us
            # descriptor generations also overlap.
            N_PAIRS = N_CHUNKS // 2
            out_t = out_pool.tile([128, OUT_W], mybir.dt.float16)
            pair_order = [0, 1, 2, 3, 4, 5, 7, 6]  # 6=(12,13) DVE-last
            for i, p in enumerate(pair_order):
                nc.tensor.matmul(
                    acc[:, :],
                    exall[:, 0, 2 * p:2 * p + 2, :],
                    exall[:, 1:5, 2 * p:2 * p + 2, :].transpose(
                        [0, 2, 1, 3]),
                    start=(i == 0), stop=(i == N_PAIRS - 1),
                    perf_mode=mybir.MatmulPerfMode.DoubleRow,
                )
            nc.vector.tensor_copy(out_t[:, :256], acc[:, :256])
            nc.sync.dma_start(out=out[:, :256], in_=out_t[:, :256])
            nc.scalar.copy(out=out_t[:, 256:], in_=acc[:, 256:])
            nc.scalar.dma_start(out=out[:, 256:], in_=out_t[:, 256:])

    nc.compile()
    return nc


_PROGRAM = None


def _get_program():
    global _PROGRAM
    if _PROGRAM is None:
        _PROGRAM = build_program()
    return _PROGRAM


def _quat2mat(q):
    q = q / np.linalg.norm(q)
    w, x, y, z = q
    return np.array([
        [1 - 2 * (y * y + z * z), 2 * (x * y - z * w), 2 * (x * z + y * w)],
        [2 * (x * y + z * w), 1 - 2 * (x * x + z * z), 2 * (y * z - x * w)],
        [2 * (x * z - y * w), 2 * (y * z + x * w), 1 - 2 * (x * x + y * y)],
    ])


def kernel(positions, colors, opacities, scales, qvec, tvec, tile_hw,
           chunk_gauss, _trace=False):
    positions = np.asarray(positions, dtype=np.float32)
    colors = np.asarray(colors, dtype=np.float32)
    opacities = np.asarray(opacities, dtype=np.float32)
    scales = np.asarray(scales, dtype=np.float32)
    qvec = np.asarray(qvec, dtype=np.float32)
    tvec = np.asarray(tvec, dtype=np.float32)
    tile_hw = int(tile_hw)
    chunk_gauss = int(chunk_gauss)
    n = positions.shape[0]
    assert n == N_GAUSS, f"expected {N_GAUSS} gaussians, got {n}"

    # ---- O(N) per-gaussian prep in float64 (rounds to the same f32 values
    # the reference computes, to well within the exp's own error budget) ----
    R = _quat2mat(qvec.astype(np.float64))
    cam = positions.astype(np.float64) @ R.T + tvec.astype(np.float64)
    ax = cam[:, 0] / cam[:, 2] * FX + CX          # [N] screen x center
    ay = cam[:, 1] / cam[:, 2] * FY + CY          # [N] screen y center
    var = scales[:, 0].astype(np.float64) ** 2
    s = -0.5 / var                                # [N] negative inv 2*var

    # centered coords keep the quadratic-expansion terms small (|u|<=64)
    dx = ax - CX
    dy = ay - CY

    bf16 = mybir.dt.np(mybir.dt.bfloat16)

    def hilo(x):
        """Split x into bf16 hi+lo with hi+lo ~= x to ~2^-17 relative."""
        hi = np.asarray(x, np.float32).astype(bf16)
        lo = (np.asarray(x, np.float64) - hi.astype(np.float64)
              ).astype(np.float32).astype(bf16)
        return hi, lo

    # K=7 bf16 stationary rows per axis per gaussian, for
    #   arg_x = s*u^2 + (-2 s dx)*u + s*dx^2     (u = x - 64)
    # expanded so every PE product is (near-)exact despite the bf16
    # 8-bit significand: s*u^2 = shi*u2hi + shi*u2lo + slo*u2hi (+O(2^-16)),
    # where u2hi+u2lo is an EXACT bf16 split of u^2 (u2lo is a small int).
    # The linear/const terms use plain hi/lo pairs against exact rows.
    # +ln(op) on the y-constant row makes exp(arg_y) = op*exp_y directly.
    # Max |arg error| ~4e-3 -> ~0.4% worst-case weight error, averages out.
    op64 = opacities[:, 0].astype(np.float64)
    axes = [(s, -2.0 * s * dx, s * dx * dx),
            (s, -2.0 * s * dy, s * dy * dy + np.log(op64))]
    coef_rows = []
    for (a, b, c) in axes:
        ahi, alo = hilo(a)
        bhi, blo = hilo(b)
        chi, clo = hilo(c)
        coef_rows.extend([ahi, ahi, alo, bhi, blo, chi, clo])
    coef_full = np.stack(coef_rows)                       # [14, N] bf16

    u = np.arange(W, dtype=np.float64) - CX
    u2 = u * u
    u2hi = u2.astype(np.float32).astype(bf16)
    u2lo = (u2 - u2hi.astype(np.float64)).astype(np.float32).astype(bf16)
    assert np.all(u2hi.astype(np.float64) + u2lo.astype(np.float64) == u2)
    zeros = np.zeros(128, dtype=bf16)
    ones = np.ones(128, dtype=bf16)
    ub = u.astype(np.float32).astype(bf16)
    axis_rows = [u2hi, u2lo, u2hi, ub, ub, ones, ones]
    rhs_rows = []
    for r in axis_rows:
        rhs_rows.append(np.concatenate([r, zeros]))
    for r in axis_rows:
        rhs_rows.append(np.concatenate([zeros, r]))
    rhsxy = np.stack(rhs_rows)                            # [14, 256] bf16

    # colnarrow[p, chunk, c] = col_c[chunk*128+p] (bf16): the DVE color
    # multiply broadcasts it along y on-chip.
    col16 = colors.astype(np.float32).astype(bf16)        # [N, 3]

    # ---- shard gaussians across the 8 cores ----
    in_maps = []
    for core in range(N_CORES):
        g0 = core * G_PER_CORE
        g1 = g0 + G_PER_CORE
        coln_c = np.ascontiguousarray(
            col16[g0:g1].reshape(N_CHUNKS, CHUNK, 3).transpose(1, 0, 2))
        cpack_c = np.ascontiguousarray(
            np.concatenate([coef_full[:, g0:g1], rhsxy], axis=1))
        in_maps.append({
            "cpack": cpack_c,
            "colnarrow": coln_c,
        })

    nc = _get_program()
    res = run_bass_kernel_spmd(nc, in_maps, list(range(N_CORES)),
                               trace=_trace)

    # ---- host reduction: sum per-core partials, divide, reshape ----
    acc = np.zeros((128, 4, 128), dtype=np.float64)   # [x, (den|r|g|b), y]
    for core in range(N_CORES):
        acc += res.results[core]["out"].astype(np.float64).reshape(128, 4, 128)

    num = acc[:, 1:4, :]                          # [x, c, y]
    n_chunks_ref = n // chunk_gauss
    den = acc[:, 0, :] + n_chunks_ref * EPS       # [x, y]
    img = num / den[:, None, :]                   # [x, c, y]
    img = img.transpose(2, 0, 1).reshape(H * W, 3)  # [p=(y,x), c]

    step = tile_hw * tile_hw
    t = (H * W) // step
    out = img.reshape(t, step, 3).transpose(0, 2, 1).reshape(
        t, 3, tile_hw, tile_hw)
    result = out.astype(np.float32)
    if _trace:
        return result, res
    return result



# revision 46
# speedup vs baseline: 1.0621x; 1.0450x over previous
"""Trainium2 Bass kernel for the isotropic-gaussian differentiable renderer.

Math: for pixel p=(x,y) and gaussian g:
    w[g,p] = op_g * exp(-0.5*((x-ax_g)^2+(y-ay_g)^2)/var_g)
    img[p,c] = (sum_g w[g,p]*col_gc) / (sum_g w[g,p] + n_chunks*EPS)

The isotropic RBF is separable: w = op * exp(sx) * exp(sy) with
sx = s*(x-ax)^2, sy = s*(y-ay)^2 + ln(op), s = -0.5/var.  That turns the
268M-element exp into 2*N*128 exps plus matmuls:

  per 128-gaussian chunk:
    PE (bf16): arg[g, 0:128]=sx(g,x), arg[g,128:256]=sy(g,y) via a K=14
               matmul against fixed rows [u2hi,u2lo,u2hi,u,u,1,1|...] with
               bf16 hi/lo coefficient splits (centered coords keep every
               product near-exact despite the cancelling quadratic;
               max |arg err| ~4e-3).  bf16 needs NO on-chip cast, so the
               args start the moment the single coef DMA lands.
    ACT      : ONE exp per group writes the x|B planes of the plane-major
               fp8 tile exall[128, 5, chunk, y] (planes expx|B|r|g|b);
               B = op*expy directly (ln(op) is in the argument)
    DVE      : one fused tensor_mul per chunk PAIR fills the r|g|b planes
               col_c*B (colnarrow broadcast along y on-chip); the LAST
               pair runs on ACT via Copy-with-scale so both engines drain
               the color work in parallel.  All consume the SAME rounded
               fp8 B, so weight rounding cancels in num/den.
    PE (fp8) : 8 DoubleRow matmuls, each contracting a chunk PAIR
               (2 k-subtiles, 2 rows/cycle):
               acc[x, (den|c)*128+y] += expx_pair^T @ [B|r|g|b]_pair
               (fp32 PSUM accumulate; PE pre-warmed off memset tiles).

Scheduling notes (hard-won):
  - engine-issued dma_start costs 0.6-1.1us of SERIAL descriptor
    generation on its sequencer, and rings deliver no data before ~8.4us:
    all inputs are packed into 2 small transfers (cpack 64KB + colnarrow
    12KB) on the sync ring.
  - interleaving two PSUM accumulation groups in one tile corrupts
    results; the mains form a single full-width group.
  - GpSimd fp8 ops and gpsimd tensor_copy crash or run ~10x slow; only
    memset rides there.

Sharding: gaussians split 2048/core across 8 cores; every core accumulates
the full 128x128 image; host sums the 8 fp16 partials in float64, divides
num/den and reshapes to the reference's [4,3,64,64] tile layout.
Measured: ~23.5us HW exec, scale-rel err ~6.8e-3 (gate 2e-2).
"""
import numpy as np

import concourse.bacc as bacc
import concourse.tile as tile
from concourse import mybir
from concourse.bass_utils import run_bass_kernel_spmd

# Problem constants (hardcoded per harness contract)
N_GAUSS = 16384
H = 128
W = 128
FX = 128.0
FY = 128.0
CX = 64.0
CY = 64.0
EPS = 1e-8
N_CORES = 8
G_PER_CORE = N_GAUSS // N_CORES      # 2048
CHUNK = 128                          # gaussians per matmul chunk
N_CHUNKS = G_PER_CORE // CHUNK       # 16
ARG_W = 256                          # per-chunk arg width: 128 x | 128 y
GROUP = 4                            # chunks per exp batch
N_GROUPS = N_CHUNKS // GROUP         # 4
OUT_W = 512                          # (c,y) free width of the accumulator

F32 = mybir.dt.float32
MM_DT = mybir.dt.float8e4            # main-accumulation matmul dtype.
# fp8e4 is safe for the same reason fp16 was: expx rounding cancels
# exactly between num and den (same lhsT), B rounding mostly cancels
# (colB multiplies the rounded B), and the residual per-gaussian color
# rounding (~6%) averages out over the ~10^3 gaussians per pixel.
# Measured scale-relative error ~7e-3 vs the 2e-2 gate.
KARG = 14                            # arg-matmul contraction: 7 bf16 coef rows/axis


def build_program():
    """One SPMD Bass program; every core runs it on its gaussian slice."""
    nc = bacc.Bacc("TRN2", target_bir_lowering=False, debug=False,
                   num_devices=N_CORES)
    # coef rows for all chunks plus the fixed moving rows, packed into ONE
    # dram tensor: every engine-issued dma_start costs ~0.6-1.1us of serial
    # descriptor generation on its sequencer, so the latency-critical arg
    # inputs must be a single transfer.
    cpack_shape = [KARG, G_PER_CORE + ARG_W]
    cpack = nc.dram_tensor("cpack", cpack_shape, mybir.dt.bfloat16,
                           kind="ExternalInput")
    # [128, 16, 3]: colnarrow[p, chunk, c] = col_c[chunk*128+p] (bf16) —
    # broadcast on-chip along y by the DVE color multiply, so the whole
    # color input is one tiny DMA instead of a 1.5MB replicated tensor.
    colnarrow = nc.dram_tensor("colnarrow", [128, N_CHUNKS, 3],
                               mybir.dt.bfloat16, kind="ExternalInput")
    # partial accumulator: [x, c*128+y], fp16 (halves the output DMA; the
    # host sums the 8 per-core partials in float64 anyway)
    out = nc.dram_tensor("out", [128, OUT_W], mybir.dt.float16,
                         kind="ExternalOutput")

    with tile.TileContext(nc) as tc:
        with tc.tile_pool(name="ins", bufs=1) as ins_pool, \
             tc.tile_pool(name="expp", bufs=1) as exp_pool, \
             tc.tile_pool(name="args", bufs=3, space="PSUM") as arg_pool, \
             tc.tile_pool(name="acc", bufs=1, space="PSUM") as acc_pool, \
             tc.tile_pool(name="outp", bufs=1) as out_pool:

            cpack_t = ins_pool.tile(cpack_shape, mybir.dt.bfloat16)
            coln_t = ins_pool.tile([128, N_CHUNKS, 3], mybir.dt.bfloat16)
            # warmup operand, memset on the early-starting GpSimd
            wsrc = ins_pool.tile([128, ARG_W], mybir.dt.bfloat16)
            nc.gpsimd.memset(wsrc, 0.0)
            # sync ring: just two small input transfers (~76KB total), so
            # every consumer's DMA wait clears by ~9us.
            nc.sync.dma_start(out=cpack_t, in_=cpack[:, :])
            nc.sync.dma_start(out=coln_t, in_=colnarrow[:, :])
            # ACT's scale operand must be f32: one cheap DVE copy converts
            # the 6 entries its chunks need (first in the DVE stream; its
            # only dependency is the tiny colnarrow DMA).
            opcf = ins_pool.tile([128, 2, 3, 1], F32)
            nc.vector.tensor_copy(opcf, coln_t[:, 14:16, :].unsqueeze(3))


            # plane-major storage [plane, chunk, y]: planes are
            # [expx | B | r | g | b].  The DoubleRow lhsT (expx plane)
            # stays chunk-packed, and ONE activation per group writes the
            # x and B planes together through a transposed AP.
            exall = exp_pool.tile([128, 5, N_CHUNKS, 128], MM_DT)
            # two accumulators in SEPARATE PSUM tiles: interleaved
            # accumulation groups are legal across tiles (tile_matmul
            # pattern), letting the [den|r] half's drain start one matmul
            # earlier and overlap the [g|b] half's finale.
            accA = acc_pool.tile([128, OUT_W // 2], F32)
            accB = acc_pool.tile([128, OUT_W // 2], F32)

            # PE warmup off the memset tile (ready ~6.5us, while inputs
            # stream in): dummy matmuls flip the HAM clock gate toward 8/8
            # so the real arg matmuls stream at 1 cyc/row, in dead time.
            # the warmup PSUM tile rotates inside the args pool (it is
            # dead before args group 2 needs its buffer) — keeps total
            # PSUM at 8 banks with the two split accumulators.
            wdst = arg_pool.tile([128, 1, ARG_W], F32, tag="args")
            for _ in range(9):
                nc.tensor.matmul(wdst[:, 0, :], wsrc[:, :CHUNK], wsrc[:, :],
                                 start=True, stop=True)

            # narrow leading groups tighten the pipeline front: chunk 0's
            # A-build waits on a 1-chunk exp instead of a 4-chunk batch
            group_plan = [(0, 1), (1, 1), (2, 2), (4, 4), (8, 4), (12, 4)]
            for g0c, width in group_plan:
                args = arg_pool.tile([128, width, ARG_W], F32, tag="args")
                for k in range(width):
                    chunk = g0c + k
                    nc.tensor.matmul(
                        args[:, k, :],
                        cpack_t[:, chunk * CHUNK:(chunk + 1) * CHUNK],
                        cpack_t[:, G_PER_CORE:G_PER_CORE + ARG_W],
                        start=True, stop=True,
                    )
                # one call writes [x|B] for the whole group: out iterates
                # (chunk, plane, y) to match the psum args layout
                nc.scalar.activation(
                    out=exall[:, 0:2, g0c:g0c + width, :].transpose(
                        [0, 2, 1, 3]),
                    in_=args[:, :, :],
                    func=mybir.ActivationFunctionType.Exp,
                )

            # color blocks multiply the SAME rounded B so num/den rounding
            # cancels.  rhs column order: [den(B)|r|g|b].  DVE: ONE fused
            # tensor_mul per chunk PAIR (B broadcast across the 3 channels,
            # colnarrow broadcast along y).  The last pair runs on ACT
            # (Copy with per-partition scale), which ends its exp stream
            # about when DVE finishes pair 6 — the engines drain the color
            # work in parallel.
            # pair 0 as two per-chunk ops so the DVE stream starts the
            # moment chunk 0's exp lands (the pair op would wait chunk 1)
            for c0, cw in [(0, 1), (1, 1), (2, 2), (4, 2), (6, 2),
                           (8, 2), (10, 2), (12, 2)]:
                nc.vector.tensor_mul(
                    exall[:, 2:5, c0:c0 + cw, :],
                    exall[:, 1:2, c0:c0 + cw, :].broadcast_to(
                        [128, 3, cw, 128]),
                    coln_t[:, c0:c0 + cw, :].transpose(
                        [0, 2, 1]).unsqueeze(3).broadcast_to(
                        [128, 3, cw, 128]),
                )
            for chunk in (14, 15):
                for c in range(3):
                    nc.scalar.mul(
                        exall[:, 2 + c, chunk, :],
                        exall[:, 1, chunk, :],
                        opcf[:, chunk - 14, c],
                    )
            # fp8 DoubleRow mains: one matmul contracts a PAIR of
            # 128-gaussian chunks (2 k-subtiles at 2 rows/cycle).  Full
            # width, single accumulation group (interleaving two groups in
            # one PSUM tile corrupts the result).  The PSUM then drains as
            # two fp16 copies on DVE+ACT in parallel, with the two output
            # doorbells on separate rings (sync/scalar) so their ~0.7us
            # descriptor generations also overlap.
            N_PAIRS = N_CHUNKS // 2
            out_t = out_pool.tile([128, OUT_W], mybir.dt.float16)
            pair_order = [0, 1, 2, 3, 4, 5, 7, 6]  # 6=(12,13) DVE-last
            for i, p in enumerate(pair_order):
                for h, acch in ((0, accA), (1, accB)):
                    nc.tensor.matmul(
                        acch[:, :],
                        exall[:, 0, 2 * p:2 * p + 2, :],
                        exall[:, 1 + 2 * h:3 + 2 * h, 2 * p:2 * p + 2, :]
                        .transpose([0, 2, 1, 3]),
                        start=(i == 0), stop=(i == N_PAIRS - 1),
                        perf_mode=mybir.MatmulPerfMode.DoubleRow,
                    )
            nc.vector.tensor_copy(out_t[:, :256], accA[:, :])
            nc.sync.dma_start(out=out[:, :256], in_=out_t[:, :256])
            nc.scalar.copy(out=out_t[:, 256:], in_=accB[:, :])
            nc.scalar.dma_start(out=out[:, 256:], in_=out_t[:, 256:])

    nc.compile()
    return nc


_PROGRAM = None


def _get_program():
    global _PROGRAM
    if _PROGRAM is None:
        _PROGRAM = build_program()
    return _PROGRAM


def _quat2mat(q):
    q = q / np.linalg.norm(q)
    w, x, y, z = q
    return np.array([
        [1 - 2 * (y * y + z * z), 2 * (x * y - z * w), 2 * (x * z + y * w)],
        [2 * (x * y + z * w), 1 - 2 * (x * x + z * z), 2 * (y * z - x * w)],
        [2 * (x * z - y * w), 2 * (y * z + x * w), 1 - 2 * (x * x + y * y)],
    ])


def kernel(positions, colors, opacities, scales, qvec, tvec, tile_hw,
           chunk_gauss, _trace=False):
    positions = np.asarray(positions, dtype=np.float32)
    colors = np.asarray(colors, dtype=np.float32)
    opacities = np.asarray(opacities, dtype=np.float32)
    scales = np.asarray(scales, dtype=np.float32)
    qvec = np.asarray(qvec, dtype=np.float32)
    tvec = np.asarray(tvec, dtype=np.float32)
    tile_hw = int(tile_hw)
    chunk_gauss = int(chunk_gauss)
    n = positions.shape[0]
    assert n == N_GAUSS, f"expected {N_GAUSS} gaussians, got {n}"

    # ---- O(N) per-gaussian prep in float64 (rounds to the same f32 values
    # the reference computes, to well within the exp's own error budget) ----
    R = _quat2mat(qvec.astype(np.float64))
    cam = positions.astype(np.float64) @ R.T + tvec.astype(np.float64)
    ax = cam[:, 0] / cam[:, 2] * FX + CX          # [N] screen x center
    ay = cam[:, 1] / cam[:, 2] * FY + CY          # [N] screen y center
    var = scales[:, 0].astype(np.float64) ** 2
    s = -0.5 / var                                # [N] negative inv 2*var

    # centered coords keep the quadratic-expansion terms small (|u|<=64)
    dx = ax - CX
    dy = ay - CY

    bf16 = mybir.dt.np(mybir.dt.bfloat16)

    def hilo(x):
        """Split x into bf16 hi+lo with hi+lo ~= x to ~2^-17 relative."""
        hi = np.asarray(x, np.float32).astype(bf16)
        lo = (np.asarray(x, np.float64) - hi.astype(np.float64)
              ).astype(np.float32).astype(bf16)
        return hi, lo

    # K=7 bf16 stationary rows per axis per gaussian, for
    #   arg_x = s*u^2 + (-2 s dx)*u + s*dx^2     (u = x - 64)
    # expanded so every PE product is (near-)exact despite the bf16
    # 8-bit significand: s*u^2 = shi*u2hi + shi*u2lo + slo*u2hi (+O(2^-16)),
    # where u2hi+u2lo is an EXACT bf16 split of u^2 (u2lo is a small int).
    # The linear/const terms use plain hi/lo pairs against exact rows.
    # +ln(op) on the y-constant row makes exp(arg_y) = op*exp_y directly.
    # Max |arg error| ~4e-3 -> ~0.4% worst-case weight error, averages out.
    op64 = opacities[:, 0].astype(np.float64)
    axes = [(s, -2.0 * s * dx, s * dx * dx),
            (s, -2.0 * s * dy, s * dy * dy + np.log(op64))]
    coef_rows = []
    for (a, b, c) in axes:
        ahi, alo = hilo(a)
        bhi, blo = hilo(b)
        chi, clo = hilo(c)
        coef_rows.extend([ahi, ahi, alo, bhi, blo, chi, clo])
    coef_full = np.stack(coef_rows)                       # [14, N] bf16

    u = np.arange(W, dtype=np.float64) - CX
    u2 = u * u
    u2hi = u2.astype(np.float32).astype(bf16)
    u2lo = (u2 - u2hi.astype(np.float64)).astype(np.float32).astype(bf16)
    assert np.all(u2hi.astype(np.float64) + u2lo.astype(np.float64) == u2)
    zeros = np.zeros(128, dtype=bf16)
    ones = np.ones(128, dtype=bf16)
    ub = u.astype(np.float32).astype(bf16)
    axis_rows = [u2hi, u2lo, u2hi, ub, ub, ones, ones]
    rhs_rows = []
    for r in axis_rows:
        rhs_rows.append(np.concatenate([r, zeros]))
    for r in axis_rows:
        rhs_rows.append(np.concatenate([zeros, r]))
    rhsxy = np.stack(rhs_rows)                            # [14, 256] bf16

    # colnarrow[p, chunk, c] = col_c[chunk*128+p] (bf16): the DVE color
    # multiply broadcasts it along y on-chip.
    col16 = colors.astype(np.float32).astype(bf16)        # [N, 3]

    # ---- shard gaussians across the 8 cores ----
    in_maps = []
    for core in range(N_CORES):
        g0 = core * G_PER_CORE
        g1 = g0 + G_PER_CORE
        coln_c = np.ascontiguousarray(
            col16[g0:g1].reshape(N_CHUNKS, CHUNK, 3).transpose(1, 0, 2))
        cpack_c = np.ascontiguousarray(
            np.concatenate([coef_full[:, g0:g1], rhsxy], axis=1))
        in_maps.append({
            "cpack": cpack_c,
            "colnarrow": coln_c,
        })

    nc = _get_program()
    res = run_bass_kernel_spmd(nc, in_maps, list(range(N_CORES)),
                               trace=_trace)

    # ---- host reduction: sum per-core partials, divide, reshape ----
    acc = np.zeros((128, 4, 128), dtype=np.float64)   # [x, (den|r|g|b), y]
    for core in range(N_CORES):
        acc += res.results[core]["out"].astype(np.float64).reshape(128, 4, 128)

    num = acc[:, 1:4, :]                          # [x, c, y]
    n_chunks_ref = n // chunk_gauss
    den = acc[:, 0, :] + n_chunks_ref * EPS       # [x, y]
    img = num / den[:, None, :]                   # [x, c, y]
    img = img.transpose(2, 0, 1).reshape(H * W, 3)  # [p=(y,x), c]

    step = tile_hw * tile_hw
    t = (H * W) // step
    out = img.reshape(t, step, 3).transpose(0, 2, 1).reshape(
        t, 3, tile_hw, tile_hw)
    result = out.astype(np.float32)
    if _trace:
        return result, res
    return result

